# revision 49
# baseline (speedup 1.0000x reference)
"""Trainium2 Bass kernel for nn_DeformableConvStandard.

The deformable interpolation + both convs are linear in `inp` once the
(tiny) offsets are known, so the whole module collapses to

    out = Pt + (Pd * sigmoid(ctrl @ W + bparam)),   Pt = X @ A_t,  Pd = X @ D

with A_t, D: [48, 12] host-built from offsets/conv weights. The device
kernel is a feature-major batched small-matmul + sigmoid + multiply-add,
data-parallel over the batch axis across 8 cores (16 batches each).

Feature-major layout: one "supertile" = one batch = 4096 rows = 8 groups
of 512 nodes. SBUF tiles are [96, 512] = 8 groups x 12 features (gate
path) or 4 groups x 24 features (x path), so matmuls contract 96
partitions and stream 512 columns; elementwise ops run 96 partitions
wide. The gate bias is accumulated into PSUM with an identity matmul
(PE is idle-ish; DVE is the scarcer engine).
"""
import numpy as np
from contextlib import ExitStack

N_CORES = 8
B, NNODES = 128, 4096
NDW, LL, NPRED, NDRIFT = 3, 16, 12, 2
BPC = B // N_CORES          # batches per core: 16
NG = NNODES // 512          # node groups per batch: 8
MM_DT_F32R = True           # use relaxed-fp32 matmul (1 cyc/row vs 4)
STREAM_FP16 = True          # stream xp/ct/bias/weights as fp16 (halves DMA)
OUT_FP16 = True             # store outputs as fp16 (2x ACT copy, half out-DMA)


def _build_A(offset, conv_w, mode):
    """A [48, 12] with pred = X @ A for X [rows, 48], feature = d*16+l."""
    off = np.asarray(offset, np.float32)
    pos = np.tanh(off) * np.float32(NDRIFT) + (
        np.arange(NPRED, dtype=np.float32) + np.float32(NDRIFT)
    )[None, :]
    key = np.floor(pos)
    frac = (pos - key).astype(np.float64)
    idx = key.astype(np.int32)
    M = np.zeros((NDW, LL, NPRED), np.float64)
    for d in range(NDW):
        for j in range(NPRED):
            M[d, idx[d, j], j] += 1.0 - frac[d, j]
            M[d, idx[d, j] + 1, j] += frac[d, j]
    A = np.zeros((NDW, LL, NPRED), np.float64)
    w = np.asarray(conv_w, np.float64)
    if mode == "t":
        for p in range(NPRED):
            for k in range(3):
                j = p + k - 1
                if 0 <= j < NPRED:
                    A[:, :, p] += w[0, :, k][:, None] * M[:, :, j]
    else:
        for o in range(NPRED):
            for d in range(NDW):
                for c in range(NPRED):
                    A[d, :, o] += w[o, c, d] * M[d, :, c]
    return A.reshape(NDW * LL, NPRED)


def _build_weight_tiles(offset_t, offset_n, conv_t_w, conv_n_w, W):
    """Stationary lhsT tiles [5, 96, 112]: blockdiag-W, bias-permutation,
    X-chunk0, X-chunk1, and I48 (final PE-side accumulate)."""
    A_t = _build_A(offset_t, conv_t_w, "t")
    D = _build_A(offset_n, conv_n_w, "n") - A_t
    # Gate-path output columns are padded to 112: groups 0-3 at partitions
    # 0-47, groups 4-7 at 64-111, so SBUF slices per half start at 0/64
    # (HW constraint: SBUF compute APs start only at 0/32/64/96).
    wts = np.zeros((5, 96, 112), np.float64)
    wts[4, 0:48, 0:48] = np.eye(48)  # I48 for the PE-side final accumulate
    Wf = np.asarray(W, np.float64)
    for j in range(8):
        col = (j // 4) * 64 + (j % 4) * 12
        wts[0, j * 12:(j + 1) * 12, col:col + 12] = Wf
        for q in range(12):
            wts[1, j * 12 + q, col + q] = 1.0  # bias permutation "identity"
    # X-path outputs likewise: Pt at partitions 0-47, Pd at 64-111.
    for c in range(2):
        for g in range(4):
            rows = slice(g * 24, (g + 1) * 24)
            wts[2 + c, rows, g * 12:(g + 1) * 12] = A_t[c * 24:(c + 1) * 24]
            wts[2 + c, rows, 64 + g * 12:64 + (g + 1) * 12] = D[c * 24:(c + 1) * 24]
    return wts.astype(np.float32), A_t, D


POOLS = dict(xp=4, ct=3, sig=6, tmp=10, ost=4, gps=2, xps=6)
DVE_ADD_KS = (0, 3)         # which of the 4 (supertile,half) units use DVE add
XP_PER_SUPERTILE = False    # split xp DMA per supertile (768KB) vs pair (1.5MB)
OUT_DMA_ENGINE = "scalar"   # engine queue for output stores
CONST_DMA_ENGINE = "sync"   # engine queue for the two const loads


def build_program(pools=None):
    import concourse.bass as bass
    import concourse.tile as tile
    from concourse import bacc, mybir
    from concourse.bass_interp import get_hw_module
    P = dict(POOLS)
    if pools:
        P.update(pools)

    if STREAM_FP16:
        dt_mm = mybir.dt.float16
    elif MM_DT_F32R:
        dt_mm = mybir.dt.float32r
    else:
        dt_mm = mybir.dt.float32
    f32 = mybir.dt.float32
    dt_out = mybir.dt.float16 if OUT_FP16 else f32

    nc = bacc.Bacc("TRN2", target_bir_lowering=False, debug=False,
                   num_devices=N_CORES)
    xp = nc.dram_tensor("xp", [BPC, 2, 2, 96, 512], dt_mm, kind="ExternalInput").ap()
    ct = nc.dram_tensor("ct", [BPC, 96, 512], dt_mm, kind="ExternalInput").ap()
    bias = nc.dram_tensor("bias", [96, 512], dt_mm, kind="ExternalInput").ap()
    wts = nc.dram_tensor("wts", [5, 96, 112], dt_mm, kind="ExternalInput").ap()
    yp = nc.dram_tensor("yp", [BPC, 96, 512], dt_out, kind="ExternalOutput").ap()

    with tile.TileContext(nc) as tc, ExitStack() as ctx:
        consts = ctx.enter_context(tc.tile_pool(name="consts", bufs=1))
        xpool = ctx.enter_context(tc.tile_pool(name="xp", bufs=P["xp"]))
        cpool = ctx.enter_context(tc.tile_pool(name="ct", bufs=P["ct"]))
        spool = ctx.enter_context(tc.tile_pool(name="sig", bufs=P["sig"]))
        tpool = ctx.enter_context(tc.tile_pool(name="tmp", bufs=P["tmp"]))
        opool = ctx.enter_context(tc.tile_pool(name="ost", bufs=P["ost"]))
        gps = ctx.enter_context(
            tc.tile_pool(name="gps", bufs=P["gps"], space=bass.MemorySpace.PSUM))
        xps = ctx.enter_context(
            tc.tile_pool(name="xps", bufs=P["xps"], space=bass.MemorySpace.PSUM))

        # consts first (strict FIFO within their ring): w_sb gates every
        # matmul, bias_sb gates the first sigmoid — they must land before the
        # bulk ct/xp stream.
        const_eng = getattr(nc, CONST_DMA_ENGINE)
        w_sb = consts.tile([96, 5 * 112], dt_mm)
        const_eng.dma_start(
            w_sb[:].rearrange("p (n f) -> p n f", n=5),
            wts.rearrange("n p f -> p n f"),
        )
        bias_sb = consts.tile([96, 512], dt_mm)
        const_eng.dma_start(bias_sb[:], bias[:])

        def w_slice(k, width=112):
            return w_sb[:, k * 112:k * 112 + width]

        for i4 in range(BPC // 4):
            # ctrl for 4 supertiles in one DMA (768 KB)
            ct_sb = cpool.tile([96, 4 * 512], dt_mm)
            nc.sync.dma_start(
                ct_sb[:].rearrange("p (b f) -> p b f", b=4),
                ct[i4 * 4:(i4 + 1) * 4].rearrange("b p f -> p b f"),
            )
            for pair in range(2):
                b0 = i4 * 4 + pair * 2
                if XP_PER_SUPERTILE:
                    # x features per supertile (768 KB each) — warms the
                    # pipeline sooner
                    xp_tiles = []
                    for bi in range(2):
                        xp_sb1 = xpool.tile([96, 4 * 512], dt_mm,
                                            name=f"xps{bi}", tag="xpt")
                        nc.sync.dma_start(
                            xp_sb1[:].rearrange("p (h c f) -> p h c f",
                                                h=2, c=2),
                            xp[b0 + bi].rearrange("h c p f -> p h c f"),
                        )
                        xp_tiles.append(xp_sb1)
                else:
                    # x features for 2 supertiles in one DMA (1.5 MB)
                    xp_sb = xpool.tile([96, 2 * 4 * 512], dt_mm)
                    nc.sync.dma_start(
                        xp_sb[:].rearrange("p (b h c f) -> p b h c f",
                                           b=2, h=2, c=2),
                        xp[b0:b0 + 2].rearrange("b h c p f -> p b h c f"),
                    )
                o_sb = opool.tile([112, 2 * 512], dt_out)
                # gate matmuls for both supertiles grouped so W then the bias
                # permutation each load once into the PE
                g_list = []
                for bi in range(2):
                    bb = pair * 2 + bi        # supertile index within the i4 block
                    g_ps = gps.tile([112, 512], f32)
                    nc.tensor.matmul(g_ps[:], w_slice(0),
                                     ct_sb[:, bb * 512:(bb + 1) * 512],
                                     start=True, stop=False)
                    g_list.append(g_ps)
                for bi in range(2):
                    nc.tensor.matmul(g_list[bi][:], w_slice(1), bias_sb[:],
                                     start=False, stop=True)
                s_list = []
                for bi in range(2):
                    s_sb = spool.tile([112, 512], f32)
                    nc.scalar.activation(
                        s_sb[:], g_list[bi][:],
                        mybir.ActivationFunctionType.Sigmoid)
                    s_list.append(s_sb)
                # x-path matmuls grouped per supertile so X0/X1 load once for
                # its two halves while keeping px-bank turnover fine-grained
                px_list = [xps.tile([112, 512], f32, name=f"px{k}", tag="px")
                           for k in range(4)]
                for bi in range(2):
                    for ci in range(2):
                        for h in range(2):
                            k = bi * 2 + h
                            if XP_PER_SUPERTILE:
                                rhs = xp_tiles[bi][:, h * 1024 + ci * 512:
                                                   h * 1024 + (ci + 1) * 512]
                            else:
                                rhs = xp_sb[:, k * 1024 + ci * 512:
                                            k * 1024 + (ci + 1) * 512]
                            nc.tensor.matmul(px_list[k][:], w_slice(2 + ci),
                                             rhs, start=(ci == 0),
                                             stop=(ci == 1))
                for k in range(4):
                    bi, h = divmod(k, 2)
                    px = px_list[k]
                    s_sb = s_list[bi]
                    o_slice = o_sb[64 * h:64 * h + 48,
                                   bi * 512:(bi + 1) * 512]
                    t_sb = tpool.tile([48, 512], dt_mm)
                    nc.vector.tensor_mul(
                        t_sb[:], px[64:112, :], s_sb[64 * h:64 * h + 48, :])
                    if k in DVE_ADD_KS:
                        # DVE-side fused add+store — keeps DVE and ACT
                        # near-equally loaded
                        nc.vector.tensor_add(o_slice, px[0:48, :], t_sb[:])
                    else:
                        # out = Pt + T via identity-matmul accumulate on PE
                        # (onto the closed Pt rows; start=False keeps data),
                        # then ACT copies the finished rows out to SBUF
                        nc.tensor.matmul(px[0:48, :],
                                         w_sb[0:48, 4 * 112:4 * 112 + 48],
                                         t_sb[:], start=False, stop=True,
                                         skip_group_check=True)
                        nc.scalar.activation(
                            o_slice, px[0:48, :],
                            mybir.ActivationFunctionType.Copy)
                # output for this supertile pair (2 DMAs of 384 KB) on a
                # non-input ring — keeps compute-gated stores from
                # head-of-line-blocking the input stream.
                out_eng = getattr(nc, OUT_DMA_ENGINE)
                out_eng.dma_start(
                    yp[b0:b0 + 2, 0:48].rearrange("b p f -> p b f"),
                    o_sb[0:48, :].rearrange("p (b f) -> p b f", b=2),
                )
                out_eng.dma_start(
                    yp[b0:b0 + 2, 48:96].rearrange("b p f -> p b f"),
                    o_sb[64:112, :].rearrange("p (b f) -> p b f", b=2),
                )

    nc.compile()
    nc.m = get_hw_module(nc.m)
    return nc


_PROGRAM = None


def _get_program():
    global _PROGRAM
    if _PROGRAM is None:
        _PROGRAM = build_program()
    return _PROGRAM


def pack_inputs(inp, ctrl, bparam, offset_t, offset_n, conv_t_w, conv_t_b,
                conv_n_w, conv_n_b, W):
    """Host-side shard + layout packing. Returns in_maps (list of 8 dicts)."""
    wts, A_t, D = _build_weight_tiles(offset_t, offset_n, conv_t_w, conv_n_w, W)
    X6 = np.asarray(inp, np.float32).reshape(B, 2, 4, 512, 2, 24)
    Xpack = np.ascontiguousarray(X6.transpose(0, 1, 4, 2, 5, 3)).reshape(
        B, 2, 2, 96, 512)
    CT = np.ascontiguousarray(
        np.asarray(ctrl, np.float32).reshape(B, NG, 512, 12).transpose(0, 1, 3, 2)
    ).reshape(B, 96, 512)
    bias_t = np.ascontiguousarray(
        np.asarray(bparam, np.float32).reshape(NG, 512, 12).transpose(0, 2, 1)
    ).reshape(96, 512)
    if STREAM_FP16:
        Xpack = Xpack.astype(np.float16)
        CT = CT.astype(np.float16)
        bias_t = bias_t.astype(np.float16)
        wts = wts.astype(np.float16)
    in_maps = []
    for c in range(N_CORES):
        sl = slice(c * BPC, (c + 1) * BPC)
        in_maps.append({
            "xp": Xpack[sl],
            "ct": CT[sl],
            "bias": bias_t,
            "wts": wts,
        })
    return in_maps


def unpack_output(results):
    """results: list of 8 dicts with 'yp' [BPC, 96, 512] -> out [B, N, 12]."""
    yp = np.concatenate([r["yp"] for r in results], axis=0)  # [B, 96, 512]
    return np.ascontiguousarray(
        yp.reshape(B, NG, 12, 512).transpose(0, 1, 3, 2)
    ).reshape(B, NNODES, NPRED)


def kernel(inp, ctrl, offset_t, offset_n, conv_t_w, conv_t_b, conv_n_w,
           conv_n_b, W, bparam):
    from concourse.bass_utils import run_bass_kernel_spmd

    nc = _get_program()
    in_maps = pack_inputs(inp, ctrl, bparam, offset_t, offset_n, conv_t_w,
                          conv_t_b, conv_n_w, conv_n_b, W)
    res = run_bass_kernel_spmd(nc, in_maps, core_ids=list(range(N_CORES)))
    out = unpack_output(res.results)
    # Conv biases are zeros in this module's init, so the device kernel omits
    # them. If ever nonzero, apply the exact correction out += ctb + Δ·S
    # (per-q constants through the gate) on the host.
    ctb = float(np.asarray(conv_t_b).reshape(-1)[0])
    cnb = np.asarray(conv_n_b, np.float32)
    if ctb != 0.0 or np.any(cnb != 0.0):
        # recompute gate on host (cheap-ish, correctness path only)
        G = np.asarray(ctrl, np.float32).reshape(B * NNODES, NPRED) @ np.asarray(
            W, np.float32)
        G += np.tile(np.asarray(bparam, np.float32), (B, 1))
        S = 1.0 / (1.0 + np.exp(-G))
        out = out + (ctb + (cnb[None, :] - ctb) * S).reshape(B, NNODES, NPRED)
    return out.astype(np.float32)


# revision 52
# speedup vs baseline: 1.0157x; 1.0157x over previous
"""Trainium2 Bass kernel for nn_DeformableConvStandard.

The deformable interpolation + both convs are linear in `inp` once the
(tiny) offsets are known, so the whole module collapses to

    out = Pt + (Pd * sigmoid(ctrl @ W + bparam)),   Pt = X @ A_t,  Pd = X @ D

with A_t, D: [48, 12] host-built from offsets/conv weights. The device
kernel is a feature-major batched small-matmul + sigmoid + multiply-add,
data-parallel over the batch axis across 8 cores (16 batches each).

Feature-major layout: one "supertile" = one batch = 4096 rows = 8 groups
of 512 nodes. SBUF tiles are [96, 512] = 8 groups x 12 features (gate
path) or 4 groups x 24 features (x path), so matmuls contract 96
partitions and stream 512 columns; elementwise ops run 96 partitions
wide. The gate bias is accumulated into PSUM with an identity matmul
(PE is idle-ish; DVE is the scarcer engine).
"""
import numpy as np
from contextlib import ExitStack

N_CORES = 8
B, NNODES = 128, 4096
NDW, LL, NPRED, NDRIFT = 3, 16, 12, 2
BPC = B // N_CORES          # batches per core: 16
NG = NNODES // 512          # node groups per batch: 8
MM_DT_F32R = True           # use relaxed-fp32 matmul (1 cyc/row vs 4)
STREAM_FP16 = True          # stream xp/ct/bias/weights as fp16 (halves DMA)
OUT_FP16 = True             # store outputs as fp16 (2x ACT copy, half out-DMA)


def _build_A(offset, conv_w, mode):
    """A [48, 12] with pred = X @ A for X [rows, 48], feature = d*16+l."""
    off = np.asarray(offset, np.float32)
    pos = np.tanh(off) * np.float32(NDRIFT) + (
        np.arange(NPRED, dtype=np.float32) + np.float32(NDRIFT)
    )[None, :]
    key = np.floor(pos)
    frac = (pos - key).astype(np.float64)
    idx = key.astype(np.int32)
    M = np.zeros((NDW, LL, NPRED), np.float64)
    for d in range(NDW):
        for j in range(NPRED):
            M[d, idx[d, j], j] += 1.0 - frac[d, j]
            M[d, idx[d, j] + 1, j] += frac[d, j]
    A = np.zeros((NDW, LL, NPRED), np.float64)
    w = np.asarray(conv_w, np.float64)
    if mode == "t":
        for p in range(NPRED):
            for k in range(3):
                j = p + k - 1
                if 0 <= j < NPRED:
                    A[:, :, p] += w[0, :, k][:, None] * M[:, :, j]
    else:
        for o in range(NPRED):
            for d in range(NDW):
                for c in range(NPRED):
                    A[d, :, o] += w[o, c, d] * M[d, :, c]
    return A.reshape(NDW * LL, NPRED)


def _build_weight_tiles(offset_t, offset_n, conv_t_w, conv_n_w, W):
    """Stationary lhsT tiles [5, 96, 112]: blockdiag-W, bias-permutation,
    X-chunk0, X-chunk1, and I48 (final PE-side accumulate)."""
    A_t = _build_A(offset_t, conv_t_w, "t")
    D = _build_A(offset_n, conv_n_w, "n") - A_t
    # Gate-path output columns are padded to 112: groups 0-3 at partitions
    # 0-47, groups 4-7 at 64-111, so SBUF slices per half start at 0/64
    # (HW constraint: SBUF compute APs start only at 0/32/64/96).
    wts = np.zeros((5, 96, 112), np.float64)
    wts[4, 0:48, 0:48] = np.eye(48)  # I48 for the PE-side final accumulate
    Wf = np.asarray(W, np.float64)
    for j in range(8):
        col = (j // 4) * 64 + (j % 4) * 12
        wts[0, j * 12:(j + 1) * 12, col:col + 12] = Wf
        for q in range(12):
            wts[1, j * 12 + q, col + q] = 1.0  # bias permutation "identity"
    # X-path outputs likewise: Pt at partitions 0-47, Pd at 64-111.
    for c in range(2):
        for g in range(4):
            rows = slice(g * 24, (g + 1) * 24)
            wts[2 + c, rows, g * 12:(g + 1) * 12] = A_t[c * 24:(c + 1) * 24]
            wts[2 + c, rows, 64 + g * 12:64 + (g + 1) * 12] = D[c * 24:(c + 1) * 24]
    return wts.astype(np.float32), A_t, D


POOLS = dict(xp=4, ct=3, sig=6, tmp=10, ost=4, gps=2, xps=6)
DVE_ADD_KS = (0, 3)         # which of the 4 (supertile,half) units use DVE add
DVE_ADD_KS_ALT = (2, 3)     # used on odd pairs (better engine interleave)
XP_PER_SUPERTILE = False    # split xp DMA per supertile (768KB) vs pair (1.5MB)
OUT_DMA_ENGINE = "scalar"   # engine queue for output stores
CONST_DMA_ENGINE = "sync"   # engine queue for the two const loads


def build_program(pools=None):
    import concourse.bass as bass
    import concourse.tile as tile
    from concourse import bacc, mybir
    from concourse.bass_interp import get_hw_module
    P = dict(POOLS)
    if pools:
        P.update(pools)

    if STREAM_FP16:
        dt_mm = mybir.dt.float16
    elif MM_DT_F32R:
        dt_mm = mybir.dt.float32r
    else:
        dt_mm = mybir.dt.float32
    f32 = mybir.dt.float32
    dt_out = mybir.dt.float16 if OUT_FP16 else f32

    nc = bacc.Bacc("TRN2", target_bir_lowering=False, debug=False,
                   num_devices=N_CORES)
    xp = nc.dram_tensor("xp", [BPC, 2, 2, 96, 512], dt_mm, kind="ExternalInput").ap()
    ct = nc.dram_tensor("ct", [BPC, 96, 512], dt_mm, kind="ExternalInput").ap()
    bias = nc.dram_tensor("bias", [96, 512], dt_mm, kind="ExternalInput").ap()
    wts = nc.dram_tensor("wts", [5, 96, 112], dt_mm, kind="ExternalInput").ap()
    yp = nc.dram_tensor("yp", [BPC, 96, 512], dt_out, kind="ExternalOutput").ap()

    with tile.TileContext(nc) as tc, ExitStack() as ctx:
        consts = ctx.enter_context(tc.tile_pool(name="consts", bufs=1))
        xpool = ctx.enter_context(tc.tile_pool(name="xp", bufs=P["xp"]))
        cpool = ctx.enter_context(tc.tile_pool(name="ct", bufs=P["ct"]))
        spool = ctx.enter_context(tc.tile_pool(name="sig", bufs=P["sig"]))
        tpool = ctx.enter_context(tc.tile_pool(name="tmp", bufs=P["tmp"]))
        opool = ctx.enter_context(tc.tile_pool(name="ost", bufs=P["ost"]))
        gps = ctx.enter_context(
            tc.tile_pool(name="gps", bufs=P["gps"], space=bass.MemorySpace.PSUM))
        xps = ctx.enter_context(
            tc.tile_pool(name="xps", bufs=P["xps"], space=bass.MemorySpace.PSUM))

        # consts first (strict FIFO within their ring): w_sb gates every
        # matmul, bias_sb gates the first sigmoid — they must land before the
        # bulk ct/xp stream.
        const_eng = getattr(nc, CONST_DMA_ENGINE)
        w_sb = consts.tile([96, 5 * 112], dt_mm)
        const_eng.dma_start(
            w_sb[:].rearrange("p (n f) -> p n f", n=5),
            wts.rearrange("n p f -> p n f"),
        )
        bias_sb = consts.tile([96, 512], dt_mm)
        const_eng.dma_start(bias_sb[:], bias[:])

        def w_slice(k, width=112):
            return w_sb[:, k * 112:k * 112 + width]

        for i4 in range(BPC // 4):
            # ctrl for 4 supertiles in one DMA (768 KB)
            ct_sb = cpool.tile([96, 4 * 512], dt_mm)
            nc.sync.dma_start(
                ct_sb[:].rearrange("p (b f) -> p b f", b=4),
                ct[i4 * 4:(i4 + 1) * 4].rearrange("b p f -> p b f"),
            )
            for pair in range(2):
                b0 = i4 * 4 + pair * 2
                if XP_PER_SUPERTILE:
                    # x features per supertile (768 KB each) — warms the
                    # pipeline sooner
                    xp_tiles = []
                    for bi in range(2):
                        xp_sb1 = xpool.tile([96, 4 * 512], dt_mm,
                                            name=f"xps{bi}", tag="xpt")
                        nc.sync.dma_start(
                            xp_sb1[:].rearrange("p (h c f) -> p h c f",
                                                h=2, c=2),
                            xp[b0 + bi].rearrange("h c p f -> p h c f"),
                        )
                        xp_tiles.append(xp_sb1)
                else:
                    # x features for 2 supertiles in one DMA (1.5 MB)
                    xp_sb = xpool.tile([96, 2 * 4 * 512], dt_mm)
                    nc.sync.dma_start(
                        xp_sb[:].rearrange("p (b h c f) -> p b h c f",
                                           b=2, h=2, c=2),
                        xp[b0:b0 + 2].rearrange("b h c p f -> p b h c f"),
                    )
                o_sb = opool.tile([112, 2 * 512], dt_out)
                # gate matmuls for both supertiles grouped so W then the bias
                # permutation each load once into the PE
                g_list = []
                for bi in range(2):
                    bb = pair * 2 + bi        # supertile index within the i4 block
                    g_ps = gps.tile([112, 512], f32)
                    nc.tensor.matmul(g_ps[:], w_slice(0),
                                     ct_sb[:, bb * 512:(bb + 1) * 512],
                                     start=True, stop=False)
                    g_list.append(g_ps)
                for bi in range(2):
                    nc.tensor.matmul(g_list[bi][:], w_slice(1), bias_sb[:],
                                     start=False, stop=True)
                s_list = []
                for bi in range(2):
                    s_sb = spool.tile([112, 512], f32)
                    nc.scalar.activation(
                        s_sb[:], g_list[bi][:],
                        mybir.ActivationFunctionType.Sigmoid)
                    s_list.append(s_sb)
                # x-path matmuls grouped per supertile so X0/X1 load once for
                # its two halves while keeping px-bank turnover fine-grained
                px_list = [xps.tile([112, 512], f32, name=f"px{k}", tag="px")
                           for k in range(4)]
                for bi in range(2):
                    for ci in range(2):
                        for h in range(2):
                            k = bi * 2 + h
                            if XP_PER_SUPERTILE:
                                rhs = xp_tiles[bi][:, h * 1024 + ci * 512:
                                                   h * 1024 + (ci + 1) * 512]
                            else:
                                rhs = xp_sb[:, k * 1024 + ci * 512:
                                            k * 1024 + (ci + 1) * 512]
                            nc.tensor.matmul(px_list[k][:], w_slice(2 + ci),
                                             rhs, start=(ci == 0),
                                             stop=(ci == 1))
                for k in range(4):
                    bi, h = divmod(k, 2)
                    px = px_list[k]
                    s_sb = s_list[bi]
                    o_slice = o_sb[64 * h:64 * h + 48,
                                   bi * 512:(bi + 1) * 512]
                    t_sb = tpool.tile([48, 512], dt_mm)
                    nc.vector.tensor_mul(
                        t_sb[:], px[64:112, :], s_sb[64 * h:64 * h + 48, :])
                    dve_ks = DVE_ADD_KS
                    if DVE_ADD_KS_ALT is not None and (i4 * 2 + pair) % 2:
                        dve_ks = DVE_ADD_KS_ALT
                    if k in dve_ks:
                        # DVE-side fused add+store — keeps DVE and ACT
                        # near-equally loaded
                        nc.vector.tensor_add(o_slice, px[0:48, :], t_sb[:])
                    else:
                        # out = Pt + T via identity-matmul accumulate on PE
                        # (onto the closed Pt rows; start=False keeps data),
                        # then ACT copies the finished rows out to SBUF
                        nc.tensor.matmul(px[0:48, :],
                                         w_sb[0:48, 4 * 112:4 * 112 + 48],
                                         t_sb[:], start=False, stop=True,
                                         skip_group_check=True)
                        nc.scalar.activation(
                            o_slice, px[0:48, :],
                            mybir.ActivationFunctionType.Copy)
                # output for this supertile pair (2 DMAs of 384 KB) on a
                # non-input ring — keeps compute-gated stores from
                # head-of-line-blocking the input stream.
                out_eng = getattr(nc, OUT_DMA_ENGINE)
                out_eng.dma_start(
                    yp[b0:b0 + 2, 0:48].rearrange("b p f -> p b f"),
                    o_sb[0:48, :].rearrange("p (b f) -> p b f", b=2),
                )
                out_eng.dma_start(
                    yp[b0:b0 + 2, 48:96].rearrange("b p f -> p b f"),
                    o_sb[64:112, :].rearrange("p (b f) -> p b f", b=2),
                )

    nc.compile()
    nc.m = get_hw_module(nc.m)
    return nc


_PROGRAM = None


def _get_program():
    global _PROGRAM
    if _PROGRAM is None:
        _PROGRAM = build_program()
    return _PROGRAM


def pack_inputs(inp, ctrl, bparam, offset_t, offset_n, conv_t_w, conv_t_b,
                conv_n_w, conv_n_b, W):
    """Host-side shard + layout packing. Returns in_maps (list of 8 dicts)."""
    wts, A_t, D = _build_weight_tiles(offset_t, offset_n, conv_t_w, conv_n_w, W)
    X6 = np.asarray(inp, np.float32).reshape(B, 2, 4, 512, 2, 24)
    Xpack = np.ascontiguousarray(X6.transpose(0, 1, 4, 2, 5, 3)).reshape(
        B, 2, 2, 96, 512)
    CT = np.ascontiguousarray(
        np.asarray(ctrl, np.float32).reshape(B, NG, 512, 12).transpose(0, 1, 3, 2)
    ).reshape(B, 96, 512)
    bias_t = np.ascontiguousarray(
        np.asarray(bparam, np.float32).reshape(NG, 512, 12).transpose(0, 2, 1)
    ).reshape(96, 512)
    if STREAM_FP16:
        Xpack = Xpack.astype(np.float16)
        CT = CT.astype(np.float16)
        bias_t = bias_t.astype(np.float16)
        wts = wts.astype(np.float16)
    in_maps = []
    for c in range(N_CORES):
        sl = slice(c * BPC, (c + 1) * BPC)
        in_maps.append({
            "xp": Xpack[sl],
            "ct": CT[sl],
            "bias": bias_t,
            "wts": wts,
        })
    return in_maps


def unpack_output(results):
    """results: list of 8 dicts with 'yp' [BPC, 96, 512] -> out [B, N, 12]."""
    yp = np.concatenate([r["yp"] for r in results], axis=0)  # [B, 96, 512]
    return np.ascontiguousarray(
        yp.reshape(B, NG, 12, 512).transpose(0, 1, 3, 2)
    ).reshape(B, NNODES, NPRED)


def kernel(inp, ctrl, offset_t, offset_n, conv_t_w, conv_t_b, conv_n_w,
           conv_n_b, W, bparam):
    from concourse.bass_utils import run_bass_kernel_spmd

    nc = _get_program()
    in_maps = pack_inputs(inp, ctrl, bparam, offset_t, offset_n, conv_t_w,
                          conv_t_b, conv_n_w, conv_n_b, W)
    res = run_bass_kernel_spmd(nc, in_maps, core_ids=list(range(N_CORES)))
    out = unpack_output(res.results)
    # Conv biases are zeros in this module's init, so the device kernel omits
    # them. If ever nonzero, apply the exact correction out += ctb + Δ·S
    # (per-q constants through the gate) on the host.
    ctb = float(np.asarray(conv_t_b).reshape(-1)[0])
    cnb = np.asarray(conv_n_b, np.float32)
    if ctb != 0.0 or np.any(cnb != 0.0):
        # recompute gate on host (cheap-ish, correctness path only)
        G = np.asarray(ctrl, np.float32).reshape(B * NNODES, NPRED) @ np.asarray(
            W, np.float32)
        G += np.tile(np.asarray(bparam, np.float32), (B, 1))
        S = 1.0 / (1.0 + np.exp(-G))
        out = out + (ctb + (cnb[None, :] - ctb) * S).reshape(B, NNODES, NPRED)
    return out.astype(np.float32)


# revision 63
# speedup vs baseline: 1.0316x; 1.0157x over previous
"""Trainium2 Bass kernel for nn_DeformableConvStandard.

The deformable interpolation + both convs are linear in `inp` once the
(tiny) offsets are known, so the whole module collapses to

    out = Pt + (Pd * sigmoid(ctrl @ W + bparam)),   Pt = X @ A_t,  Pd = X @ D

with A_t, D: [48, 12] host-built from offsets/conv weights. The device
kernel is a feature-major batched small-matmul + sigmoid + multiply-add,
data-parallel over the batch axis across 8 cores (16 batches each).

Feature-major layout: one "supertile" = one batch = 4096 rows = 8 groups
of 512 nodes. SBUF tiles are [96, 512] = 8 groups x 12 features (gate
path) or 4 groups x 24 features (x path), so matmuls contract 96
partitions and stream 512 columns; elementwise ops run 96 partitions
wide. The gate bias is accumulated into PSUM with an identity matmul
(PE is idle-ish; DVE is the scarcer engine).
"""
import numpy as np
from contextlib import ExitStack

N_CORES = 8
B, NNODES = 128, 4096
NDW, LL, NPRED, NDRIFT = 3, 16, 12, 2
BPC = B // N_CORES          # batches per core: 16
NG = NNODES // 512          # node groups per batch: 8
MM_DT_F32R = True           # use relaxed-fp32 matmul (1 cyc/row vs 4)
STREAM_FP16 = True          # stream xp/ct/bias/weights as fp16 (halves DMA)
OUT_FP16 = True             # store outputs as fp16 (2x ACT copy, half out-DMA)


def _build_A(offset, conv_w, mode):
    """A [48, 12] with pred = X @ A for X [rows, 48], feature = d*16+l."""
    off = np.asarray(offset, np.float32)
    pos = np.tanh(off) * np.float32(NDRIFT) + (
        np.arange(NPRED, dtype=np.float32) + np.float32(NDRIFT)
    )[None, :]
    key = np.floor(pos)
    frac = (pos - key).astype(np.float64)
    idx = key.astype(np.int32)
    M = np.zeros((NDW, LL, NPRED), np.float64)
    for d in range(NDW):
        for j in range(NPRED):
            M[d, idx[d, j], j] += 1.0 - frac[d, j]
            M[d, idx[d, j] + 1, j] += frac[d, j]
    A = np.zeros((NDW, LL, NPRED), np.float64)
    w = np.asarray(conv_w, np.float64)
    if mode == "t":
        for p in range(NPRED):
            for k in range(3):
                j = p + k - 1
                if 0 <= j < NPRED:
                    A[:, :, p] += w[0, :, k][:, None] * M[:, :, j]
    else:
        for o in range(NPRED):
            for d in range(NDW):
                for c in range(NPRED):
                    A[d, :, o] += w[o, c, d] * M[d, :, c]
    return A.reshape(NDW * LL, NPRED)


def _build_weight_tiles(offset_t, offset_n, conv_t_w, conv_n_w, W):
    """Stationary lhsT tiles [5, 96, 112]: blockdiag-W, bias-permutation,
    X-chunk0, X-chunk1, and I48 (final PE-side accumulate)."""
    A_t = _build_A(offset_t, conv_t_w, "t")
    D = _build_A(offset_n, conv_n_w, "n") - A_t
    # Gate-path output columns are padded to 112: groups 0-3 at partitions
    # 0-47, groups 4-7 at 64-111, so SBUF slices per half start at 0/64
    # (HW constraint: SBUF compute APs start only at 0/32/64/96).
    wts = np.zeros((5, 96, 112), np.float64)
    wts[4, 0:48, 0:48] = np.eye(48)  # I48 for the PE-side final accumulate
    Wf = np.asarray(W, np.float64)
    for j in range(8):
        col = (j // 4) * 64 + (j % 4) * 12
        wts[0, j * 12:(j + 1) * 12, col:col + 12] = Wf
        for q in range(12):
            wts[1, j * 12 + q, col + q] = 1.0  # bias permutation "identity"
    # X-path outputs likewise: Pt at partitions 0-47, Pd at 64-111.
    for c in range(2):
        for g in range(4):
            rows = slice(g * 24, (g + 1) * 24)
            wts[2 + c, rows, g * 12:(g + 1) * 12] = A_t[c * 24:(c + 1) * 24]
            wts[2 + c, rows, 64 + g * 12:64 + (g + 1) * 12] = D[c * 24:(c + 1) * 24]
    return wts.astype(np.float32), A_t, D


POOLS = dict(xp=5, ct=3, sig=6, tmp=10, ost=4, gps=2, xps=6)
DVE_ADD_KS = (0, 3)         # which of the 4 (supertile,half) units use DVE add
DVE_ADD_KS_ALT = (2, 3)     # used on odd pairs (better engine interleave)
XP_PER_SUPERTILE = False    # split xp DMA per supertile (768KB) vs pair (1.5MB)
FINE_XP_PAIRS = (0,)        # pairs (of 8) forced to per-supertile xp DMAs
CT_SPLIT = 1                # ct DMAs per 4-supertile block (1, 2, or 4)
N_WARM_MM = 0               # dummy PE warm-up matmuls before the real stream
OUT_DMA_ENGINE = "scalar"   # engine queue for output stores
CONST_DMA_ENGINE = "sync"   # engine queue for the two const loads


def build_program(pools=None):
    import concourse.bass as bass
    import concourse.tile as tile
    from concourse import bacc, mybir
    from concourse.bass_interp import get_hw_module
    P = dict(POOLS)
    if pools:
        P.update(pools)

    if STREAM_FP16:
        dt_mm = mybir.dt.float16
    elif MM_DT_F32R:
        dt_mm = mybir.dt.float32r
    else:
        dt_mm = mybir.dt.float32
    f32 = mybir.dt.float32
    dt_out = mybir.dt.float16 if OUT_FP16 else f32

    nc = bacc.Bacc("TRN2", target_bir_lowering=False, debug=False,
                   num_devices=N_CORES)
    xp = nc.dram_tensor("xp", [BPC, 2, 2, 96, 512], dt_mm, kind="ExternalInput").ap()
    ct = nc.dram_tensor("ct", [BPC, 96, 512], dt_mm, kind="ExternalInput").ap()
    bias = nc.dram_tensor("bias", [96, 512], dt_mm, kind="ExternalInput").ap()
    wts = nc.dram_tensor("wts", [5, 96, 112], dt_mm, kind="ExternalInput").ap()
    yp = nc.dram_tensor("yp", [BPC, 96, 512], dt_out, kind="ExternalOutput").ap()

    with tile.TileContext(nc) as tc, ExitStack() as ctx:
        consts = ctx.enter_context(tc.tile_pool(name="consts", bufs=1))
        xpool = ctx.enter_context(tc.tile_pool(name="xp", bufs=P["xp"]))
        cpool = ctx.enter_context(tc.tile_pool(name="ct", bufs=P["ct"]))
        spool = ctx.enter_context(tc.tile_pool(name="sig", bufs=P["sig"]))
        tpool = ctx.enter_context(tc.tile_pool(name="tmp", bufs=P["tmp"]))
        opool = ctx.enter_context(tc.tile_pool(name="ost", bufs=P["ost"]))
        gps = ctx.enter_context(
            tc.tile_pool(name="gps", bufs=P["gps"], space=bass.MemorySpace.PSUM))
        xps = ctx.enter_context(
            tc.tile_pool(name="xps", bufs=P["xps"], space=bass.MemorySpace.PSUM))

        # consts first (strict FIFO within their ring): w_sb gates every
        # matmul, bias_sb gates the first sigmoid — they must land before the
        # bulk ct/xp stream.
        const_eng = getattr(nc, CONST_DMA_ENGINE)
        w_sb = consts.tile([96, 5 * 112], dt_mm)
        # gate weights (W + bias permutation) land first so the gate pipeline
        # starts ~1.3us earlier; X weights follow (xp data arrives later
        # anyway)
        const_eng.dma_start(
            w_sb[:, 0:2 * 112].rearrange("p (n f) -> p n f", n=2),
            wts[0:2].rearrange("n p f -> p n f"),
        )
        bias_sb = consts.tile([96, 512], dt_mm)
        const_eng.dma_start(bias_sb[:], bias[:])
        const_eng.dma_start(
            w_sb[:, 2 * 112:].rearrange("p (n f) -> p n f", n=3),
            wts[2:5].rearrange("n p f -> p n f"),
        )

        def w_slice(k, width=112):
            return w_sb[:, k * 112:k * 112 + width]

        if N_WARM_MM:
            # PE p-state/HAM warm-up: dummy matmuls on memset scratch keep
            # the PE activity monitor busy while the first input DMAs land,
            # so the real matmul stream starts at full clock.
            warm_sb = consts.tile([96, 512], dt_mm)
            nc.gpsimd.memset(warm_sb[:], 0.0)
            wp = gps.tile([112, 512], f32, name="warmps", tag="g_ps")
            for _ in range(N_WARM_MM):
                nc.tensor.matmul(wp[:], warm_sb[:, 0:112], warm_sb[:],
                                 start=True, stop=True)

        for i4 in range(BPC // 4):
            # ctrl for 4 supertiles (one DMA, or split for earlier arrival)
            ct_sb = cpool.tile([96, 4 * 512], dt_mm)
            if CT_SPLIT == 1:
                nc.sync.dma_start(
                    ct_sb[:].rearrange("p (b f) -> p b f", b=4),
                    ct[i4 * 4:(i4 + 1) * 4].rearrange("b p f -> p b f"),
                )
            else:
                step = 4 // CT_SPLIT
                for j in range(CT_SPLIT):
                    c0 = i4 * 4 + j * step
                    nc.sync.dma_start(
                        ct_sb[:, j * step * 512:(j + 1) * step * 512]
                        .rearrange("p (b f) -> p b f", b=step),
                        ct[c0:c0 + step].rearrange("b p f -> p b f"),
                    )
            for pair in range(2):
                b0 = i4 * 4 + pair * 2
                gpair = i4 * 2 + pair
                fine = XP_PER_SUPERTILE or gpair in FINE_XP_PAIRS
                if fine:
                    # x features per supertile (768 KB each) — warms the
                    # pipeline sooner
                    xp_tiles = []
                    for bi in range(2):
                        xp_sb1 = xpool.tile([96, 4 * 512], dt_mm,
                                            name=f"xps{bi}", tag="xpt")
                        nc.sync.dma_start(
                            xp_sb1[:].rearrange("p (h c f) -> p h c f",
                                                h=2, c=2),
                            xp[b0 + bi].rearrange("h c p f -> p h c f"),
                        )
                        xp_tiles.append(xp_sb1)
                else:
                    # x features for 2 supertiles in one DMA (1.5 MB)
                    xp_sb = xpool.tile([96, 2 * 4 * 512], dt_mm)
                    nc.sync.dma_start(
                        xp_sb[:].rearrange("p (b h c f) -> p b h c f",
                                           b=2, h=2, c=2),
                        xp[b0:b0 + 2].rearrange("b h c p f -> p b h c f"),
                    )
                o_sb = opool.tile([112, 2 * 512], dt_out)
                # gate matmuls for both supertiles grouped so W then the bias
                # permutation each load once into the PE
                g_list = []
                for bi in range(2):
                    bb = pair * 2 + bi        # supertile index within the i4 block
                    g_ps = gps.tile([112, 512], f32)
                    nc.tensor.matmul(g_ps[:], w_slice(0),
                                     ct_sb[:, bb * 512:(bb + 1) * 512],
                                     start=True, stop=False)
                    g_list.append(g_ps)
                for bi in range(2):
                    nc.tensor.matmul(g_list[bi][:], w_slice(1), bias_sb[:],
                                     start=False, stop=True)
                s_list = []
                for bi in range(2):
                    s_sb = spool.tile([112, 512], f32)
                    nc.scalar.activation(
                        s_sb[:], g_list[bi][:],
                        mybir.ActivationFunctionType.Sigmoid)
                    s_list.append(s_sb)
                # x-path matmuls grouped per supertile so X0/X1 load once for
                # its two halves while keeping px-bank turnover fine-grained
                px_list = [xps.tile([112, 512], f32, name=f"px{k}", tag="px")
                           for k in range(4)]
                for bi in range(2):
                    for ci in range(2):
                        for h in range(2):
                            k = bi * 2 + h
                            if fine:
                                rhs = xp_tiles[bi][:, h * 1024 + ci * 512:
                                                   h * 1024 + (ci + 1) * 512]
                            else:
                                rhs = xp_sb[:, k * 1024 + ci * 512:
                                            k * 1024 + (ci + 1) * 512]
                            nc.tensor.matmul(px_list[k][:], w_slice(2 + ci),
                                             rhs, start=(ci == 0),
                                             stop=(ci == 1))
                for k in range(4):
                    bi, h = divmod(k, 2)
                    px = px_list[k]
                    s_sb = s_list[bi]
                    o_slice = o_sb[64 * h:64 * h + 48,
                                   bi * 512:(bi + 1) * 512]
                    t_sb = tpool.tile([48, 512], dt_mm)
                    nc.vector.tensor_mul(
                        t_sb[:], px[64:112, :], s_sb[64 * h:64 * h + 48, :])
                    dve_ks = DVE_ADD_KS
                    if DVE_ADD_KS_ALT is not None and (i4 * 2 + pair) % 2:
                        dve_ks = DVE_ADD_KS_ALT
                    if k in dve_ks:
                        # DVE-side fused add+store — keeps DVE and ACT
                        # near-equally loaded
                        nc.vector.tensor_add(o_slice, px[0:48, :], t_sb[:])
                    else:
                        # out = Pt + T via identity-matmul accumulate on PE
                        # (onto the closed Pt rows; start=False keeps data),
                        # then ACT copies the finished rows out to SBUF
                        nc.tensor.matmul(px[0:48, :],
                                         w_sb[0:48, 4 * 112:4 * 112 + 48],
                                         t_sb[:], start=False, stop=True,
                                         skip_group_check=True)
                        nc.scalar.activation(
                            o_slice, px[0:48, :],
                            mybir.ActivationFunctionType.Copy)
                # output for this supertile pair (2 DMAs of 384 KB) on a
                # non-input ring — keeps compute-gated stores from
                # head-of-line-blocking the input stream.
                out_eng = getattr(nc, OUT_DMA_ENGINE)
                out_eng.dma_start(
                    yp[b0:b0 + 2, 0:48].rearrange("b p f -> p b f"),
                    o_sb[0:48, :].rearrange("p (b f) -> p b f", b=2),
                )
                out_eng.dma_start(
                    yp[b0:b0 + 2, 48:96].rearrange("b p f -> p b f"),
                    o_sb[64:112, :].rearrange("p (b f) -> p b f", b=2),
                )

    nc.compile()
    nc.m = get_hw_module(nc.m)
    return nc


_PROGRAM = None


def _get_program():
    global _PROGRAM
    if _PROGRAM is None:
        _PROGRAM = build_program()
    return _PROGRAM


def pack_inputs(inp, ctrl, bparam, offset_t, offset_n, conv_t_w, conv_t_b,
                conv_n_w, conv_n_b, W):
    """Host-side shard + layout packing. Returns in_maps (list of 8 dicts)."""
    wts, A_t, D = _build_weight_tiles(offset_t, offset_n, conv_t_w, conv_n_w, W)
    X6 = np.asarray(inp, np.float32).reshape(B, 2, 4, 512, 2, 24)
    Xpack = np.ascontiguousarray(X6.transpose(0, 1, 4, 2, 5, 3)).reshape(
        B, 2, 2, 96, 512)
    CT = np.ascontiguousarray(
        np.asarray(ctrl, np.float32).reshape(B, NG, 512, 12).transpose(0, 1, 3, 2)
    ).reshape(B, 96, 512)
    bias_t = np.ascontiguousarray(
        np.asarray(bparam, np.float32).reshape(NG, 512, 12).transpose(0, 2, 1)
    ).reshape(96, 512)
    if STREAM_FP16:
        Xpack = Xpack.astype(np.float16)
        CT = CT.astype(np.float16)
        bias_t = bias_t.astype(np.float16)
        wts = wts.astype(np.float16)
    in_maps = []
    for c in range(N_CORES):
        sl = slice(c * BPC, (c + 1) * BPC)
        in_maps.append({
            "xp": Xpack[sl],
            "ct": CT[sl],
            "bias": bias_t,
            "wts": wts,
        })
    return in_maps


def unpack_output(results):
    """results: list of 8 dicts with 'yp' [BPC, 96, 512] -> out [B, N, 12]."""
    yp = np.concatenate([r["yp"] for r in results], axis=0)  # [B, 96, 512]
    return np.ascontiguousarray(
        yp.reshape(B, NG, 12, 512).transpose(0, 1, 3, 2)
    ).reshape(B, NNODES, NPRED)


def kernel(inp, ctrl, offset_t, offset_n, conv_t_w, conv_t_b, conv_n_w,
           conv_n_b, W, bparam):
    from concourse.bass_utils import run_bass_kernel_spmd

    nc = _get_program()
    in_maps = pack_inputs(inp, ctrl, bparam, offset_t, offset_n, conv_t_w,
                          conv_t_b, conv_n_w, conv_n_b, W)
    res = run_bass_kernel_spmd(nc, in_maps, core_ids=list(range(N_CORES)))
    out = unpack_output(res.results)
    # Conv biases are zeros in this module's init, so the device kernel omits
    # them. If ever nonzero, apply the exact correction out += ctb + Δ·S
    # (per-q constants through the gate) on the host.
    ctb = float(np.asarray(conv_t_b).reshape(-1)[0])
    cnb = np.asarray(conv_n_b, np.float32)
    if ctb != 0.0 or np.any(cnb != 0.0):
        # recompute gate on host (cheap-ish, correctness path only)
        G = np.asarray(ctrl, np.float32).reshape(B * NNODES, NPRED) @ np.asarray(
            W, np.float32)
        G += np.tile(np.asarray(bparam, np.float32), (B, 1))
        S = 1.0 / (1.0 + np.exp(-G))
        out = out + (ctb + (cnb[None, :] - ctb) * S).reshape(B, NNODES, NPRED)
    return out.astype(np.float32)
